# revision 4
# baseline (speedup 1.0000x reference)
"""Contrastive-loss kernel for Trainium2 (8 NeuronCores, SPMD, raw Bass).

loss = sum_{i != j} dist[i,j] / (2 N (N-1)) with
dist[i,j] = ||x_i||^2 + ||y_j||^2 - 2 x_i . y_j.

The full off-diagonal sum collapses algebraically:
    sum_{i!=j} dist = (N-1)*(Sx + Sy) + 2*tr - 2 * sx . sy
with Sx = sum_i ||x_i||^2, sx = sum_i x_i (column sums), tr = sum_i x_i.y_i.
Only Sx+Sy (combined), tr, and the column sums sx, sy are needed, all
O(N*D) reductions. Each core reads its 1/8 row-shard of both tensors and
returns per-partition partials; the host combines them in float64.

v3 design:
  - Host casts both shards to bf16 and concatenates them into one
    [128, 4096] SBUF image (partition p holds rows 8p..8p+7 of x then y).
    bf16 halves HBM traffic (memory-bound regime) and doubles DVE/ACT
    throughput; the bf16 rounding noise is ~1e-5 relative on the loss.
  - 2 chunk DMAs (x, y) on SP's ring; one monotone semaphore (a ring
    completes in order).
  - ACT: ONE Square activation over the whole [128, 4096] image with
    accum_out -> per-partition Sx+Sy partial (table prewarmed in the
    DMA shadow).
  - DVE: strided tensor_reduce over [p, d, k] views for x / y column
    sums + one fused tensor_tensor_reduce for tr. No PE, no Pool, no
    PSUM.
  - One [128, 260] f32 output DMA; host collapses partitions in f64.
"""

import numpy as np

N, D = 8192, 128
NCORES = 8
ROWS = N // NCORES          # 1024 rows per core per tensor
P = 128                     # SBUF partitions
KG = ROWS // P              # 8 row-groups folded into the free dim
FREE = KG * D               # 1024 free elements per partition per tensor
OUTW = 260

_NC_CACHE = {}


def _build_bass():
    from contextlib import ExitStack

    import concourse.bass as bass
    from concourse import mybir

    f32 = mybir.dt.float32
    bf16 = mybir.dt.bfloat16
    SQ = mybir.ActivationFunctionType.Square
    MUL = mybir.AluOpType.mult
    ADD = mybir.AluOpType.add
    AXX = mybir.AxisListType.X

    nc = bass.Bass()
    xy = nc.dram_tensor("xy", [P, 2 * FREE], bf16, kind="ExternalInput")
    out = nc.dram_tensor("out", [P, OUTW], f32, kind="ExternalOutput")

    with ExitStack() as ctx:
        XY = ctx.enter_context(nc.sbuf_tensor("XY", [P, 2 * FREE], bf16))
        scr_a = ctx.enter_context(nc.sbuf_tensor("scr_a", [P, 2 * FREE], bf16))
        scr_m = ctx.enter_context(nc.sbuf_tensor("scr_m", [P, FREE], bf16))
        warm = ctx.enter_context(nc.sbuf_tensor("warm", [P, 1], bf16))
        outsb = ctx.enter_context(nc.sbuf_tensor("outsb", [P, OUTW], f32))

        din = ctx.enter_context(nc.semaphore("din"))
        done = ctx.enter_context(nc.semaphore("done"))
        dout = ctx.enter_context(nc.semaphore("dout"))

        X = XY[:, 0:FREE]
        Y = XY[:, FREE:2 * FREE]

        def kview(ap):  # [p, (k d)] -> [p, d, k] so axis=X reduces over k
            return ap.rearrange("p (k d) -> p d k", d=D)

        with nc.Block() as block:

            @block.sync
            def _(sync):
                sync.dma_start(out=XY[:, 0:FREE],
                               in_=xy[:, 0:FREE]).then_inc(din, 16)
                sync.dma_start(out=XY[:, FREE:2 * FREE],
                               in_=xy[:, FREE:2 * FREE]).then_inc(din, 16)
                sync.wait_ge(done, 2)
                sync.dma_start(out=out[:, :], in_=outsb[:]).then_inc(dout, 16)
                sync.wait_ge(dout, 16)

            @block.scalar
            def _(scalar):
                # Prewarm the Square PWP table while the DMAs fly.
                nc.scalar.activation(out=warm[:], in_=warm[:], func=SQ)
                scalar.wait_ge(din, 32)
                nc.scalar.activation(out=scr_a[:], in_=XY[:], func=SQ,
                                     accum_out=outsb[:, 256:257])
                # Trailing copy orders the auto-emitted accumulator read
                # before the done increment.
                nc.scalar.copy(out=outsb[0:1, 258:259],
                               in_=outsb[0:1, 256:257]).then_inc(done, 1)

            @block.vector
            def _(vector):
                vector.wait_ge(din, 16)
                nc.vector.reduce_sum(outsb[:, 0:D], kview(X), axis=AXX)
                vector.wait_ge(din, 32)
                nc.vector.scalar_tensor_tensor(
                    out=scr_m[:], in0=X, scalar=1.0, in1=Y,
                    op0=MUL, op1=MUL, accum_out=outsb[:, 257:258])
                nc.vector.reduce_sum(outsb[:, D:2 * D], kview(Y),
                                     axis=AXX).then_inc(done, 1)

    return nc


def _get_nc():
    if "nc" not in _NC_CACHE:
        _NC_CACHE["nc"] = _build_bass()
    return _NC_CACHE["nc"]


def _make_inputs(f1, f2):
    import ml_dtypes

    bf = ml_dtypes.bfloat16
    in_maps = []
    for c in range(NCORES):
        xs = f1[c * ROWS:(c + 1) * ROWS].reshape(P, FREE)
        ys = f2[c * ROWS:(c + 1) * ROWS].reshape(P, FREE)
        in_maps.append({"xy": np.ascontiguousarray(
            np.concatenate([xs, ys], axis=1).astype(bf))})
    return in_maps


def _run_device(f1, f2, **spmd_kwargs):
    from concourse.bass_utils import run_bass_kernel_spmd

    nc = _get_nc()
    in_maps = _make_inputs(f1, f2)
    return run_bass_kernel_spmd(nc, in_maps, core_ids=list(range(NCORES)),
                                **spmd_kwargs)


def _combine(results):
    sx = np.zeros(D, np.float64)
    sy = np.zeros(D, np.float64)
    Sxy = tr = 0.0
    for r in results:
        o = r["out"].astype(np.float64)
        sx += o[:, 0:D].sum(axis=0)
        sy += o[:, D:2 * D].sum(axis=0)
        Sxy += o[:, 256].sum()
        tr += o[:, 257].sum()
    total = (N - 1) * Sxy + 2.0 * tr - 2.0 * float(sx @ sy)
    loss = total / 2.0 / (N * (N - 1))
    return np.asarray(loss, dtype=np.float32)


def kernel(feature1, feature2, label=None, **_unused):
    f1 = np.ascontiguousarray(np.asarray(feature1, dtype=np.float32))
    f2 = np.ascontiguousarray(np.asarray(feature2, dtype=np.float32))
    res = _run_device(f1, f2)
    return _combine(res.results)


# revision 7
# speedup vs baseline: 1.4673x; 1.4673x over previous
"""Contrastive-loss kernel for Trainium2 (8 NeuronCores, SPMD, raw Bass).

loss = sum_{i != j} dist[i,j] / (2 N (N-1)) with
dist[i,j] = ||x_i||^2 + ||y_j||^2 - 2 x_i . y_j.

The full off-diagonal sum collapses algebraically:
    sum_{i!=j} dist = (N-1)*(Sx + Sy) + 2*tr - 2 * sx . sy
with Sx = sum_i ||x_i||^2, sx = sum_i x_i (column sums), tr = sum_i x_i.y_i.
Each core reads its 1/8 row-shard of both tensors and returns tiny
partials; the host combines them in float64.

v4 design:
  - Host casts both shards to bf16 and concatenates them into one
    [128, 4096] SBUF image (partition p holds rows 8p..8p+7 of x then y).
    bf16 halves HBM traffic (memory-bound regime); rounding noise is
    ~1e-5 relative on the loss.
  - 2 chunk DMAs (x, y) on SP's ring; one monotone semaphore.
  - ACT: Square activations with accum_out -> per-partition Sx, Sy.
  - DVE: one fused scalar_tensor_tensor multiply with accum -> tr;
    afterwards DVE issues the PSUM->DRAM column-sum DMA.
  - PE: bf16 ones^T matmuls, 8 accumulating [128,128] matmuls per
    tensor into one PSUM bank -> fully partition-collapsed column sums
    [1, 256]; PE is p-state-prewarmed with dummy matmuls in the DMA
    shadow.
  - No final semaphore wait on the output DMAs: the runtime drains DMA
    queues at NEFF exit, so the transfers hide under the fixed
    framework epilogue.
"""

import numpy as np

N, D = 8192, 128
NCORES = 8
ROWS = N // NCORES          # 1024 rows per core per tensor
P = 128                     # SBUF partitions
KG = ROWS // P              # 8 row-groups folded into the free dim
FREE = KG * D               # 1024 free elements per partition per tensor
OUTW = 260
NWARM = 12                  # PE p-state prewarm matmuls

_NC_CACHE = {}


def _build_bass():
    from contextlib import ExitStack

    import concourse.bass as bass
    from concourse import mybir

    f32 = mybir.dt.float32
    bf16 = mybir.dt.bfloat16
    SQ = mybir.ActivationFunctionType.Square
    MUL = mybir.AluOpType.mult

    nc = bass.Bass()
    xy = nc.dram_tensor("xy", [P, 2 * FREE], bf16, kind="ExternalInput")
    out = nc.dram_tensor("out", [P, OUTW], f32, kind="ExternalOutput")

    ones = nc.const_aps.tensor(1.0, (P, 1), bf16)

    with ExitStack() as ctx:
        XY = ctx.enter_context(nc.sbuf_tensor("XY", [P, 2 * FREE], bf16))
        scr_a = ctx.enter_context(nc.sbuf_tensor("scr_a", [P, FREE], bf16))
        scr_m = ctx.enter_context(nc.sbuf_tensor("scr_m", [P, FREE], bf16))
        warm = ctx.enter_context(nc.sbuf_tensor("warm", [P, 1], bf16))
        outsb = ctx.enter_context(nc.sbuf_tensor("outsb", [P, OUTW], f32))
        ps = ctx.enter_context(nc.psum_tensor([1, 2 * D], f32))
        psw = ctx.enter_context(nc.psum_tensor([1, D], f32))

        din = ctx.enter_context(nc.semaphore("din"))
        done = ctx.enter_context(nc.semaphore("done"))
        pe_done = ctx.enter_context(nc.semaphore("pe_done"))
        dout = ctx.enter_context(nc.semaphore("dout"))

        X = XY[:, 0:FREE]
        Y = XY[:, FREE:2 * FREE]

        with nc.Block() as block:

            @block.sync
            def _(sync):
                sync.dma_start(out=XY[:, 0:FREE],
                               in_=xy[:, 0:FREE]).then_inc(din, 16)
                sync.dma_start(out=XY[:, FREE:2 * FREE],
                               in_=xy[:, FREE:2 * FREE]).then_inc(din, 16)
                sync.wait_ge(done, 3)
                sync.dma_start(out=out[:, :], in_=outsb[:]).then_inc(dout, 16)

            @block.scalar
            def _(scalar):
                # Prewarm the Square PWP table while the DMAs fly.
                nc.scalar.activation(out=warm[:], in_=warm[:], func=SQ)
                scalar.wait_ge(din, 16)
                nc.scalar.activation(out=scr_a[:], in_=X, func=SQ,
                                     accum_out=outsb[:, 0:1])
                scalar.wait_ge(din, 32)
                nc.scalar.activation(out=scr_a[:], in_=Y, func=SQ,
                                     accum_out=outsb[:, 2:3])
                # Trailing copy orders the auto-emitted accumulator reads
                # before the done increment.
                nc.scalar.copy(out=outsb[0:1, 3:4],
                               in_=outsb[0:1, 0:1]).then_inc(done, 1)

            @block.vector
            def _(vector):
                vector.wait_ge(din, 32)
                nc.vector.scalar_tensor_tensor(
                    out=scr_m[:], in0=X, scalar=1.0, in1=Y,
                    op0=MUL, op1=MUL, accum_out=outsb[:, 1:2])
                nc.vector.tensor_copy(out=outsb[0:1, 3:4],
                                      in_=outsb[0:1, 1:2]).then_inc(done, 1)
                vector.wait_ge(pe_done, 1)
                nc.vector.tensor_copy(out=outsb[0:1, 4:4 + 2 * D],
                                      in_=ps[:]).then_inc(done, 1)

            @block.tensor
            def _(tensor):
                # p-state prewarm on garbage data into a scratch bank.
                for _i in range(NWARM):
                    nc.tensor.matmul(psw[:], ones, XY[:, 0:D],
                                     start=True, stop=True)
                tensor.wait_ge(din, 16)
                for k in range(KG):
                    nc.tensor.matmul(ps[0:1, 0:D], ones,
                                     X[:, k * D:(k + 1) * D],
                                     start=(k == 0), stop=(k == KG - 1))
                tensor.wait_ge(din, 32)
                for k in range(KG):
                    mm = nc.tensor.matmul(ps[0:1, D:2 * D], ones,
                                          Y[:, k * D:(k + 1) * D],
                                          start=(k == 0), stop=(k == KG - 1))
                mm.then_inc(pe_done, 1)

    return nc


def _get_nc():
    if "nc" not in _NC_CACHE:
        _NC_CACHE["nc"] = _build_bass()
    return _NC_CACHE["nc"]


def _make_inputs(f1, f2):
    import ml_dtypes

    bf = ml_dtypes.bfloat16
    in_maps = []
    for c in range(NCORES):
        xs = f1[c * ROWS:(c + 1) * ROWS].reshape(P, FREE)
        ys = f2[c * ROWS:(c + 1) * ROWS].reshape(P, FREE)
        in_maps.append({"xy": np.ascontiguousarray(
            np.concatenate([xs, ys], axis=1).astype(bf))})
    return in_maps


def _run_device(f1, f2, **spmd_kwargs):
    from concourse.bass_utils import run_bass_kernel_spmd

    nc = _get_nc()
    in_maps = _make_inputs(f1, f2)
    return run_bass_kernel_spmd(nc, in_maps, core_ids=list(range(NCORES)),
                                **spmd_kwargs)


def _combine(results):
    sx = np.zeros(D, np.float64)
    sy = np.zeros(D, np.float64)
    Sxy = tr = 0.0
    for r in results:
        o = r["out"].astype(np.float64)
        sx += o[0, 4:4 + D]
        sy += o[0, 4 + D:4 + 2 * D]
        Sxy += o[:, 0].sum() + o[:, 2].sum()
        tr += o[:, 1].sum()
    total = (N - 1) * Sxy + 2.0 * tr - 2.0 * float(sx @ sy)
    loss = total / 2.0 / (N * (N - 1))
    return np.asarray(loss, dtype=np.float32)


def kernel(feature1, feature2, label=None, **_unused):
    f1 = np.ascontiguousarray(np.asarray(feature1, dtype=np.float32))
    f2 = np.ascontiguousarray(np.asarray(feature2, dtype=np.float32))
    res = _run_device(f1, f2)
    return _combine(res.results)
